# revision 1
# baseline (speedup 1.0000x reference)
"""Causal self-attention (B=4, T=2048, C=1024, H=16) on 8 Trainium2 NeuronCores.

Core index = 2*batch + head_group: each core owns one batch element and 8 of
the 16 heads (tensor-parallel split of c_attn output dim / c_proj input dim).
Each core emits a partial projection out^T [C, T]; the host sums the two
head-group partials per batch and adds the bias terms.

fp16 datapath (fp32 PSUM accumulation everywhere, fp32 softmax denominator):
  x, W_qk, W_v, W_p are cast to fp16 on the host. fp16 weights get FWL
  (fast weight load), making per-matmul LDWEIGHTS ~4x cheaper than fp32/f32r,
  and x^T comes from a single XBAR DMA-transpose instead of 128 PE transposes.

Per-core pipeline (Tile-scheduled, phases overlap via data deps):
  A: xT = DMA-transpose(x)                       [fp16]
  B: qkT[co, tn] = W_qk^T x^T; v = x @ W_v       [fp16 matmuls, fp32 psum]
  C per head h, per 512-wide i-chunk ic:
     S^T[j, i] = k_h^T q_h   (psum groups of 2 j-tiles [128, 2, 512])
     P = exp(S^T / 8)        (one ACT op per group -> fp16)
     causal mask on diagonal groups (DVE, precomputed mask tiles)
     U'^T [65, i] (+)= [v|1]^T P^T  over j-tiles (ones column => rowsum row 64)
     yT[hd, i] = U'^T[0:64] * bcast(1/rowsum)  (ACT copies, gpsimd
                 partition_broadcast, DVE reciprocal + multiply) -> fp16
  D: out^T = W_p^T yT -> fp32 psum -> ACT copy -> DMA
"""

import numpy as np

import concourse.bass as bass
import concourse.mybir as mybir
import concourse.tile as tile
from concourse import bacc, bass_utils

B, T, C, H = 4, 2048, 1024, 16
HD = C // H          # 64 head dim
N_CORES = 8
HG = H // 2          # 8 heads per core
CL = HG * HD         # 512 local width of q/k/v
TT = T // 128        # 16 t-tiles
CB = C // 128        # 8 c-tiles
DB = CL // 128       # 4 local-hd tiles
NIC = T // 512       # i-chunks (4)

f32 = mybir.dt.float32
f16 = mybir.dt.float16

_PROG_CACHE = {}


def _emit(tc, aps):
    nc = tc.nc
    Exp = mybir.ActivationFunctionType.Exp

    x_ap = aps["x"]
    wqk_ap = aps["wqk"]
    wv_ap = aps["wv"]
    wp_ap = aps["wp"]
    bqk_ap = aps["bqk"]
    masks_ap = aps["masks"]
    outT_ap = aps["outT"]

    from contextlib import ExitStack

    with ExitStack() as outer:
        const = outer.enter_context(tc.tile_pool(name="const", bufs=1))
        p_xT = outer.enter_context(tc.tile_pool(name="xT", bufs=1))
        p_qkT = outer.enter_context(tc.tile_pool(name="qkT", bufs=1))
        p_v = outer.enter_context(tc.tile_pool(name="vv", bufs=1))
        p_yT = outer.enter_context(tc.tile_pool(name="yT", bufs=1))
        p_w = outer.enter_context(tc.tile_pool(name="wsb", bufs=1))

        # critical-path DMAs on sync/HWDGE: wqk then x chunks
        wqk_sb = p_w.tile([128, CB, CB * 128], f16)  # [c-part, cb, co*128+q]
        nc.sync.dma_start(wqk_sb[:], wqk_ap.rearrange("(cb p) n -> p cb n", p=128))
        xT = p_xT.tile([128, CB, T], f16)
        for tn in range(NIC):
            nc.sync.dma_start_transpose(
                xT[:, :, tn * 512 : (tn + 1) * 512],
                x_ap[tn * 512 : (tn + 1) * 512, :],
            )
        wv_sb = p_w.tile([128, CB, CL], f16)
        nc.sync.dma_start(wv_sb[:], wv_ap.rearrange("(cb p) n -> p cb n", p=128))
        wp_sb = p_w.tile([128, DB, C], f16)
        nc.sync.dma_start(wp_sb[:], wp_ap.rearrange("(db p) c -> p db c", p=128))
        masks = const.tile([128, 4, 512], f16)   # 1 where j > i (to be masked)
        nc.gpsimd.dma_start(masks[:], masks_ap)
        negI = const.tile([128, 128], f16)
        nc.gpsimd.dma_start(negI[:], aps["negI"])
        bqk = const.tile([128, CB], f32)
        nc.gpsimd.dma_start(bqk[:], bqk_ap.rearrange("co p -> p co"))

        # per-(co, tn) qkT tiles, per-jt v' tiles, per-tn yT tiles
        qkT = {}
        for co in range(CB):
            for tn in range(NIC):
                qkT[(co, tn)] = p_qkT.tile(
                    [128, 512], f16, tag=f"qkT_{co}_{tn}", name=f"qkT_{co}_{tn}"
                )
        vv = {}
        for jt in range(TT):
            vv[jt] = p_v.tile([128, HG, HD + 1], f16, tag=f"vv_{jt}", name=f"vv_{jt}")
            nc.vector.memset(vv[jt][:, :, HD : HD + 1], 1.0)
        yTn = {}
        for tn in range(NIC):
            yTn[tn] = p_yT.tile([128, DB, 512], f16, tag=f"yT_{tn}", name=f"yT_{tn}")

        with ExitStack() as s_all:
            ps_ab = ExitStack()
            ps_mm = ps_ab.enter_context(tc.tile_pool(name="ps_mm", bufs=4, space="PSUM"))

            # ---- B: qkv projections, tn-major so attention can start early ---
            for tn in range(NIC):
                for co in range(CB):
                    ps = ps_mm.tile([128, 512], f32, tag="mm")
                    for cb in range(CB):
                        nc.tensor.matmul(
                            ps[:],
                            wqk_sb[:, cb, co * 128 : (co + 1) * 128],
                            xT[:, cb, tn * 512 : (tn + 1) * 512],
                            start=(cb == 0),
                            stop=(cb == CB - 1),
                        )
                    nc.vector.tensor_scalar_add(qkT[(co, tn)][:], ps[:], bqk[:, co : co + 1])
                for u in range(4):
                    tt = tn * 4 + u
                    ps = ps_mm.tile([128, CL], f32, tag="mm")
                    for cb in range(CB):
                        nc.tensor.matmul(
                            ps[:],
                            xT[:, cb, tt * 128 : (tt + 1) * 128],
                            wv_sb[:, cb, :],
                            start=(cb == 0),
                            stop=(cb == CB - 1),
                        )
                    nc.scalar.activation(
                        vv[tt][:, :, 0:HD],
                        ps.rearrange("p (h d) -> p h d", d=HD),
                        mybir.ActivationFunctionType.Copy,
                    )

            ps_ab.close()  # free A/B psum banks

            # ---- C: attention + interleaved projection -----------------------
            p_p = s_all.enter_context(tc.tile_pool(name="pp", bufs=12))
            p_usb = s_all.enter_context(tc.tile_pool(name="usb", bufs=3))
            p_rb = s_all.enter_context(tc.tile_pool(name="rb", bufs=3))
            p_ost = s_all.enter_context(tc.tile_pool(name="ost", bufs=4))
            ps_sc = s_all.enter_context(tc.tile_pool(name="ps_sc", bufs=3, space="PSUM"))
            ps_u = s_all.enter_context(tc.tile_pool(name="ps_u", bufs=2, space="PSUM"))

            def normalize(h, ic, up):
                """yT[h, ic] = U'[0:64] / rowsum."""
                poff = 64 * (h % 2)
                usb = p_usb.tile([HD, 512], f32, tag="usb", name="usb")
                nc.vector.tensor_copy(usb[:], up[0:HD, :])
                rs = p_rb.tile([1, 512], f32, tag="rs", name="rs")
                nc.vector.tensor_copy(rs[:], up[HD : HD + 1, :])
                rr = p_rb.tile([1, 512], f32, tag="rr", name="rr")
                nc.vector.reciprocal_approx_fast(rr[:], rs[:])
                rb = p_rb.tile([HD, 512], f32, tag="rb", name="rb")
                nc.gpsimd.partition_broadcast(rb[:], rr[0:1, :], channels=HD)
                nc.vector.tensor_mul(
                    yTn[ic][poff : poff + HD, h // 2, :], usb[:], rb[:]
                )

            def emit_proj(tns, cos):
                """out^T tiles for finished i-chunks; always-ready PE filler."""
                for co in cos:
                    psp = ps_sc.tile([128, 2, 512], f32, tag="sc", name="psp")
                    for ix, tn in enumerate(tns):
                        for db in range(DB):
                            nc.tensor.matmul(
                                psp[:, ix, :],
                                wp_sb[:, db, co * 128 : (co + 1) * 128],
                                yTn[tn][:, db, :],
                                start=(db == 0),
                                stop=(db == DB - 1),
                            )
                    ot = p_ost.tile([128, 2, 512], f32, tag="ot")
                    nc.vector.tensor_copy(ot[:], psp[:])
                    for ix, tn in enumerate(tns):
                        nc.sync.dma_start(
                            outT_ap[co * 128 : (co + 1) * 128, tn * 512 : (tn + 1) * 512],
                            ot[:, ix, :],
                        )

            for icp in range(NIC // 2):
                ics = [2 * icp, 2 * icp + 1]
                for h in range(HG):
                    poff = 64 * (h % 2)
                    co_q = h // 2
                    co_k = 4 + h // 2
                    ups = {
                        ic: ps_u.tile([HD + 1, 512], f32, tag="u", name=f"u_{ic}")
                        for ic in ics
                    }
                    # all (jt, ic) sub-tiles in jt-major order, packed in pairs
                    subs = [
                        (jt, ic)
                        for jt in range(4 * (ics[-1] + 1))
                        for ic in ics
                        if 4 * (ic + 1) > jt
                    ]
                    for g0 in range(0, len(subs), 2):
                        grp = subs[g0 : g0 + 2]
                        psg = ps_sc.tile([128, 2, 512], f32, tag="sc")
                        for ix, (jt, ic) in enumerate(grp):
                            m = jt % 4
                            diag = ic == jt // 4
                            lo = 128 * m if diag else 0
                            nc.tensor.matmul(
                                psg[:, ix, lo:512],
                                qkT[(co_k, jt // 4)][
                                    poff : poff + 64, m * 128 : (m + 1) * 128
                                ],
                                qkT[(co_q, ic)][poff : poff + 64, lo:512],
                                start=True,
                                stop=not diag,
                            )
                            if diag:  # -60000 above the diagonal -> exp == 0
                                nc.tensor.matmul(
                                    psg[:, ix, lo : lo + 128],
                                    negI[:],
                                    masks[:, m, lo : lo + 128],
                                    start=False,
                                    stop=True,
                                )
                        pt = p_p.tile([128, 2, 512], f16, tag="p")
                        nv = len(grp)
                        nc.scalar.activation(
                            pt[:, 0:nv, :], psg[:, 0:nv, :], Exp, scale=1.0 / np.sqrt(HD)
                        )
                        for ix, (jt, ic) in enumerate(grp):
                            m = jt % 4
                            diag = ic == jt // 4
                            lo = 128 * m if diag else 0
                            nc.tensor.matmul(
                                ups[ic][:, lo:512],
                                vv[jt][:, h, :],
                                pt[:, ix, lo:512],
                                start=(jt == 0),
                                stop=(jt == 4 * ic + 3),
                            )
                            if jt == 4 * ic + 3:
                                normalize(h, ic, ups[ic])
                    if icp > 0:
                        # previous icp's projection, one co per head: PE filler
                        emit_proj([2 * icp - 2, 2 * icp - 1], [h])
            emit_proj([NIC - 2, NIC - 1], range(CB))


def _build_program():
    nc = bacc.Bacc("TRN2", target_bir_lowering=False, debug=False, num_devices=N_CORES)
    aps = {
        "x": nc.dram_tensor("x", [T, C], f16, kind="ExternalInput").ap(),
        "wqk": nc.dram_tensor("wqk", [C, CB * 128], f16, kind="ExternalInput").ap(),
        "wv": nc.dram_tensor("wv", [C, CL], f16, kind="ExternalInput").ap(),
        "wp": nc.dram_tensor("wp", [CL, C], f16, kind="ExternalInput").ap(),
        "bqk": nc.dram_tensor("bqk", [CB, 128], f32, kind="ExternalInput").ap(),
        "masks": nc.dram_tensor("masks", [128, 4, 512], f16, kind="ExternalInput").ap(),
        "negI": nc.dram_tensor("negI", [128, 128], f16, kind="ExternalInput").ap(),
        "outT": nc.dram_tensor("outT", [C, T], f32, kind="ExternalOutput").ap(),
    }
    with tile.TileContext(nc) as tc:
        _emit(tc, aps)
    nc.compile()
    return nc


def get_program():
    if "nc" not in _PROG_CACHE:
        _PROG_CACHE["nc"] = _build_program()
    return _PROG_CACHE["nc"]


def _host_consts():
    j = np.arange(128)[:, None]
    i = np.arange(512)[None, :]
    masks = np.zeros((128, 4, 512), np.float16)
    for m in range(4):
        masks[:, m, :] = (j > i - 128 * m).astype(np.float16)  # 1 => mask out
    negI = (-60000.0 * np.eye(128)).astype(np.float16)
    return masks, negI


def make_in_maps(x, W_attn, b_attn, W_proj):
    """Build the 8 per-core input maps. Core index = 2*batch + head_group."""
    masks, negI = _host_consts()
    in_maps = []
    for core in range(N_CORES):
        b = core // 2
        g = core % 2
        wq = W_attn[:, g * CL : (g + 1) * CL]
        wk = W_attn[:, C + g * CL : C + (g + 1) * CL]
        wqk = np.concatenate([wq, wk], axis=1)  # [C, 1024], cols = co*128+q
        wv = W_attn[:, 2 * C + g * CL : 2 * C + (g + 1) * CL]
        bqk = np.concatenate(
            [b_attn[g * CL : (g + 1) * CL], b_attn[C + g * CL : C + (g + 1) * CL]]
        ).reshape(CB, 128)
        in_maps.append(
            {
                "x": np.ascontiguousarray(x[b]).astype(np.float16),
                "wqk": np.ascontiguousarray(wqk).astype(np.float16),
                "wv": np.ascontiguousarray(wv).astype(np.float16),
                "wp": np.ascontiguousarray(W_proj[g * CL : (g + 1) * CL, :]).astype(
                    np.float16
                ),
                "bqk": np.ascontiguousarray(bqk).astype(np.float32),
                "masks": masks,
                "negI": negI,
            }
        )
    return in_maps


def run(x, W_attn, b_attn, W_proj, b_proj, trace=False):
    nc = get_program()
    in_maps = make_in_maps(x, W_attn, b_attn, W_proj)
    res = bass_utils.run_bass_kernel_spmd(
        nc, in_maps, core_ids=list(range(N_CORES)), trace=trace
    )
    # combine: out[b] = sum_g outT_{2b+g}^T + (bv_g @ Wp_g summed) + b_proj
    corr = b_proj.astype(np.float64).copy()
    for g in range(2):
        bv_g = b_attn[2 * C + g * CL : 2 * C + (g + 1) * CL]
        corr += bv_g.astype(np.float64) @ W_proj[g * CL : (g + 1) * CL, :].astype(
            np.float64
        )
    out = np.empty((B, T, C), np.float32)
    for b in range(B):
        acc = (
            res.results[2 * b]["outT"].T.astype(np.float64)
            + res.results[2 * b + 1]["outT"].T.astype(np.float64)
            + corr
        )
        out[b] = acc.astype(np.float32)
    return out, res


def kernel(x, W_attn, b_attn, W_proj, b_proj):
    x = np.asarray(x, np.float32)
    W_attn = np.asarray(W_attn, np.float32)
    b_attn = np.asarray(b_attn, np.float32)
    W_proj = np.asarray(W_proj, np.float32)
    b_proj = np.asarray(b_proj, np.float32)
    out, _ = run(x, W_attn, b_attn, W_proj, b_proj)
    return out



# revision 4
# speedup vs baseline: 1.1766x; 1.1766x over previous
"""Causal self-attention (B=4, T=2048, C=1024, H=16) on 8 Trainium2 NeuronCores.

Core index = 2*batch + head_group: each core owns one batch element and 8 of
the 16 heads (tensor-parallel split of c_attn output dim / c_proj input dim).
Each core emits a partial projection out^T [C, T] in fp16; the host sums the
two head-group partials per batch and adds the bias terms.

fp16 datapath (fp32 PSUM accumulation, fp32 softmax denominator).

Schedule (single emission-ordered stream; Tile preserves per-engine order):
  B(tn) units: qkT[co-pair] = W_qk^T x^T (+bias, DVE); v = x @ W_v (ACT copy)
  C groups (ic, hp, jt): head-PAIR processing — heads 2hp (partitions 0:64)
     and 2hp+1 (64:128) issue row-tiled S^T matmuls back-to-back so they run
     CONCURRENTLY on the PE (K=64 each, tile_position (0,0)/(64,0) auto).
     One exp ACT covers both heads, trimmed to [lo:512] on diagonal blocks;
     causal masking via post-exp DVE multiply with a constant tri tile.
     PV matmuls of group g-1 are emitted after S of group g (software
     pipelining) so the PE never waits on the ACT exp.
  B(tn+1) and D projection units are interleaved into the C stream by an
     ACT-vs-PE deficit counter: the C stream alone is ACT-bound ~1.7x, so
     independent full-array matmuls fill the PE and keep HAM at K=8/8.
  D: out^T = W_p^T yT -> fp32 psum -> DVE fp16 copy -> DMA out.
"""

import numpy as np

import concourse.bass as bass
import concourse.mybir as mybir
import concourse.tile as tile
from concourse import bacc, bass_utils

B, T, C, H = 4, 2048, 1024, 16
HD = C // H          # 64 head dim
N_CORES = 8
HG = H // 2          # 8 heads per core
CL = HG * HD         # 512 local width of q/k/v
TT = T // 128        # 16 t-tiles
CB = C // 128        # 8 c-tiles
DB = CL // 128       # 4 local-hd tiles
NIC = T // 512       # i-chunks (4)

f32 = mybir.dt.float32
f16 = mybir.dt.float16

_PROG_CACHE = {}


def _emit(tc, aps):
    nc = tc.nc
    Exp = mybir.ActivationFunctionType.Exp
    Copy = mybir.ActivationFunctionType.Copy

    from contextlib import ExitStack

    with ExitStack() as outer:
        const = outer.enter_context(tc.tile_pool(name="const", bufs=1))
        p_xT = outer.enter_context(tc.tile_pool(name="xT", bufs=1))
        p_w = outer.enter_context(tc.tile_pool(name="wsb", bufs=1))
        p_qkT = outer.enter_context(tc.tile_pool(name="qkT", bufs=1))
        p_v = outer.enter_context(tc.tile_pool(name="vv", bufs=1))
        p_yT = outer.enter_context(tc.tile_pool(name="yT", bufs=1))
        p_pt = outer.enter_context(tc.tile_pool(name="pt", bufs=4))
        p_ot = outer.enter_context(tc.tile_pool(name="ot", bufs=3))
        p_r = outer.enter_context(tc.tile_pool(name="rpool", bufs=4))
        ps = outer.enter_context(tc.tile_pool(name="ps", bufs=2, space="PSUM"))
        ps_u = outer.enter_context(tc.tile_pool(name="psu", bufs=4, space="PSUM"))

        # ---- input DMAs: first-needed first; consts on the gpsimd queue ----
        wqk_sb = p_w.tile([128, CB, CB * 128], f16)  # [c-part, cb, co*128+q]
        nc.sync.dma_start(
            wqk_sb[:, :, 0:256], aps["wqk0"].rearrange("(cb p) n -> p cb n", p=128)
        )
        xT = p_xT.tile([128, CB, T], f16)
        nc.sync.dma_start_transpose(xT[:, :, 0:512], aps["x"][0:512, :])
        nc.sync.dma_start(
            wqk_sb[:, :, 256:1024], aps["wqk1"].rearrange("(cb p) n -> p cb n", p=128)
        )
        wv_sb = p_w.tile([128, CB, CL], f16)
        nc.sync.dma_start(wv_sb[:], aps["wv"].rearrange("(cb p) n -> p cb n", p=128))
        for tn in range(1, NIC):
            nc.sync.dma_start_transpose(
                xT[:, :, tn * 512 : (tn + 1) * 512],
                aps["x"][tn * 512 : (tn + 1) * 512, :],
            )
        wp_sb = p_w.tile([128, DB, C], f16)
        nc.sync.dma_start(wp_sb[:], aps["wp"].rearrange("(db p) c -> p db c", p=128))
        tri = const.tile([128, 2, 128], f16)  # keep-mask: 1 where row <= col
        nc.gpsimd.dma_start(tri[:], aps["tri"])
        bqk = const.tile([128, CB], f32)
        nc.gpsimd.dma_start(bqk[:], aps["bqk"].rearrange("co p -> p co"))

        qkT = {
            (co, tn): p_qkT.tile([128, 512], f16, tag=f"qkT_{co}_{tn}", name=f"qkT_{co}_{tn}")
            for co in range(CB)
            for tn in range(NIC)
        }
        vv = {}
        for jt in range(TT):
            vv[jt] = p_v.tile([128, HG, HD + 1], f16, tag=f"vv_{jt}", name=f"vv_{jt}")
            nc.vector.memset(vv[jt][:, :, HD : HD + 1], 1.0)
        yTn = {tn: p_yT.tile([128, DB, 512], f16, tag=f"yT_{tn}", name=f"yT_{tn}") for tn in range(NIC)}

        # ------------- emission units -------------
        def emit_qk_unit(tn, co0):
            """qkT tiles for co0, co0+1 at i-chunk tn (16 MMs + 2 DVE adds)."""
            g = ps.tile([128, 2, 512], f32, tag="g", name="g")
            for ix in range(2):
                co = co0 + ix
                for cb in range(CB):
                    nc.tensor.matmul(
                        g[:, ix, :],
                        wqk_sb[:, cb, co * 128 : (co + 1) * 128],
                        xT[:, cb, tn * 512 : (tn + 1) * 512],
                        start=(cb == 0),
                        stop=(cb == CB - 1),
                    )
            for ix in range(2):
                co = co0 + ix
                nc.vector.tensor_scalar_add(
                    qkT[(co, tn)][:], g[:, ix, :], bqk[:, co : co + 1]
                )

        def emit_v_unit(tn, u):
            """vv tiles for t-tiles 4*tn+2u, +1 (16 MMs + 2 ACT copies)."""
            g = ps.tile([128, 2, 512], f32, tag="g", name="g")
            for ix in range(2):
                tt = 4 * tn + 2 * u + ix
                for cb in range(CB):
                    nc.tensor.matmul(
                        g[:, ix, :],
                        xT[:, cb, tt * 128 : (tt + 1) * 128],
                        wv_sb[:, cb, :],
                        start=(cb == 0),
                        stop=(cb == CB - 1),
                    )
            for ix in range(2):
                tt = 4 * tn + 2 * u + ix
                nc.scalar.activation(
                    vv[tt][:, :, 0:HD],
                    g[:, ix, :].rearrange("p (h d) -> p h d", d=HD),
                    Copy,
                )

        def emit_proj_unit(tp, co):
            """out^T rows co*128.. for i-chunks 2tp, 2tp+1 (8 MMs + copy + DMA)."""
            g = ps.tile([128, 2, 512], f32, tag="g", name="g")
            for ix in range(2):
                tn = 2 * tp + ix
                for db in range(DB):
                    nc.tensor.matmul(
                        g[:, ix, :],
                        wp_sb[:, db, co * 128 : (co + 1) * 128],
                        yTn[tn][:, db, :],
                        start=(db == 0),
                        stop=(db == DB - 1),
                    )
            ot = p_ot.tile([128, 2, 512], f16, tag="ot", name="ot")
            nc.vector.tensor_copy(ot[:], g[:])
            for ix in range(2):
                tn = 2 * tp + ix
                nc.sync.dma_start(
                    aps["outT"][co * 128 : (co + 1) * 128, tn * 512 : (tn + 1) * 512],
                    ot[:, ix, :],
                )

        def emit_normalize(hp, ic, u, poff):
            rs = p_r.tile([1, 512], f32, tag="rs", name="rs")
            nc.vector.tensor_copy(rs[:], u[HD : HD + 1, :])
            rr = p_r.tile([1, 512], f32, tag="rr", name="rr")
            nc.vector.reciprocal_approx_fast(rr[:], rs[:])
            rb = p_r.tile([HD, 512], f32, tag="rb", name="rb")
            nc.gpsimd.partition_broadcast(rb[:], rr[0:1, :], channels=HD)
            nc.vector.tensor_mul(yTn[ic][poff : poff + HD, hp, :], u[0:HD, :], rb[:])

        def emit_group(ic, hp, jt, uA, uB):
            """S^T for head pair (2hp, 2hp+1) at (jt, ic); returns PV closure."""
            co_q, co_k = hp, 4 + hp
            m = jt % 4
            diag = jt // 4 == ic
            lo = 128 * m if diag else 0
            kt = jt // 4
            g = ps.tile([128, 2, 512], f32, tag="g", name="g")
            nc.tensor.matmul(
                g[:, 0, lo:512],
                qkT[(co_k, kt)][0:64, m * 128 : (m + 1) * 128],
                qkT[(co_q, ic)][0:64, lo:512],
                start=True,
                stop=True,
            )
            nc.tensor.matmul(
                g[:, 1, lo:512],
                qkT[(co_k, kt)][64:128, m * 128 : (m + 1) * 128],
                qkT[(co_q, ic)][64:128, lo:512],
                start=True,
                stop=True,
            )
            pt = p_pt.tile([128, 2, 512], f16, tag="pt", name="pt")
            nc.scalar.activation(
                pt[:, 0:2, lo:512], g[:, 0:2, lo:512], Exp, scale=1.0 / np.sqrt(HD)
            )
            if diag:
                nc.vector.tensor_mul(
                    pt[:, 0:2, lo : lo + 128], pt[:, 0:2, lo : lo + 128], tri[:]
                )

            def pv():
                nc.tensor.matmul(
                    uA[:, lo:512],
                    vv[jt][:, 2 * hp, :],
                    pt[:, 0, lo:512],
                    start=(jt == 0),
                    stop=(jt == 4 * ic + 3),
                )
                nc.tensor.matmul(
                    uB[:, lo:512],
                    vv[jt][:, 2 * hp + 1, :],
                    pt[:, 1, lo:512],
                    start=(jt == 0),
                    stop=(jt == 4 * ic + 3),
                )
                if jt == 4 * ic + 3:
                    emit_normalize(hp, ic, uA, 0)
                    emit_normalize(hp, ic, uB, 64)

            w = 512 - lo
            act_ns = (2 * w + 352) / 1.2
            pe_ns = w / 2.4 + 60 + 430
            return pv, act_ns - pe_ns

        # ------------- the schedule -------------
        # B(0): order so C(0) hp0/hp1 unlock first
        emit_qk_unit(0, 0)
        emit_qk_unit(0, 4)
        emit_v_unit(0, 0)
        emit_v_unit(0, 1)
        emit_qk_unit(0, 2)
        emit_qk_unit(0, 6)

        filler = []  # (pe_cost_ns, key, fn) in emission-feasible order
        state = {"deficit": 0.0, "pending": None}

        def run_pending():
            if state["pending"] is not None:
                state["pending"]()
                state["pending"] = None

        def pull_filler():
            while filler and state["deficit"] >= filler[0][0]:
                pe_cost, _, fn = filler.pop(0)
                fn()
                state["deficit"] -= pe_cost

        def flush_key(key):
            kept = []
            for item in filler:
                if item[1] == key:
                    item[2]()
                else:
                    kept.append(item)
            filler[:] = kept
            state["deficit"] = 0.0

        for ic in range(NIC):
            flush_key(("B", ic))
            if ic + 1 < NIC:
                tn = ic + 1
                for co0 in (0, 4, 2, 6):
                    filler.append(
                        (3600, ("B", tn), lambda t=tn, c=co0: emit_qk_unit(t, c))
                    )
                for u in (0, 1):
                    filler.append((3600, ("B", tn), lambda t=tn, uu=u: emit_v_unit(t, uu)))
            if ic == 2:
                for co in range(CB):
                    filler.append((1820, ("P", 0), lambda c=co: emit_proj_unit(0, c)))
            for hp in range(4):
                uA = ps_u.tile([HD + 1, 512], f32, tag="u", name="uA")
                uB = ps_u.tile([HD + 1, 512], f32, tag="u", name="uB")
                for jt in range(4 * (ic + 1)):
                    pv, deficit_delta = emit_group(ic, hp, jt, uA, uB)
                    run_pending()
                    pull_filler()
                    state["pending"] = pv
                    state["deficit"] += deficit_delta
            run_pending()
        flush_key(("P", 0))
        for co in range(CB):
            emit_proj_unit(1, co)


def _build_program():
    nc = bacc.Bacc("TRN2", target_bir_lowering=False, debug=False, num_devices=N_CORES)
    aps = {
        "x": nc.dram_tensor("x", [T, C], f16, kind="ExternalInput").ap(),
        "wqk0": nc.dram_tensor("wqk0", [C, 256], f16, kind="ExternalInput").ap(),
        "wqk1": nc.dram_tensor("wqk1", [C, 768], f16, kind="ExternalInput").ap(),
        "wv": nc.dram_tensor("wv", [C, CL], f16, kind="ExternalInput").ap(),
        "wp": nc.dram_tensor("wp", [CL, C], f16, kind="ExternalInput").ap(),
        "bqk": nc.dram_tensor("bqk", [CB, 128], f32, kind="ExternalInput").ap(),
        "tri": nc.dram_tensor("tri", [128, 2, 128], f16, kind="ExternalInput").ap(),
        "outT": nc.dram_tensor("outT", [C, T], f16, kind="ExternalOutput").ap(),
    }
    with tile.TileContext(nc) as tc:
        _emit(tc, aps)
    nc.compile()
    return nc


def get_program():
    if "nc" not in _PROG_CACHE:
        _PROG_CACHE["nc"] = _build_program()
    return _PROG_CACHE["nc"]


def _host_consts():
    r = np.arange(128)[:, None]
    c = np.arange(128)[None, :]
    tri = (r <= c).astype(np.float16)  # keep-mask within a diagonal 128-block
    tri2 = np.stack([tri, tri], axis=1)  # [128, 2, 128]
    return np.ascontiguousarray(tri2)


def make_in_maps(x, W_attn, b_attn, W_proj):
    """Build the 8 per-core input maps. Core index = 2*batch + head_group."""
    tri2 = _host_consts()
    in_maps = []
    for core in range(N_CORES):
        b = core // 2
        g = core % 2
        wq = W_attn[:, g * CL : (g + 1) * CL]
        wk = W_attn[:, C + g * CL : C + (g + 1) * CL]
        wqk = np.concatenate([wq, wk], axis=1).astype(np.float16)  # [C, 1024]
        wv = W_attn[:, 2 * C + g * CL : 2 * C + (g + 1) * CL]
        bqk = np.concatenate(
            [b_attn[g * CL : (g + 1) * CL], b_attn[C + g * CL : C + (g + 1) * CL]]
        ).reshape(CB, 128)
        in_maps.append(
            {
                "x": np.ascontiguousarray(x[b]).astype(np.float16),
                "wqk0": np.ascontiguousarray(wqk[:, 0:256]),
                "wqk1": np.ascontiguousarray(wqk[:, 256:1024]),
                "wv": np.ascontiguousarray(wv).astype(np.float16),
                "wp": np.ascontiguousarray(W_proj[g * CL : (g + 1) * CL, :]).astype(
                    np.float16
                ),
                "bqk": np.ascontiguousarray(bqk).astype(np.float32),
                "tri": tri2,
            }
        )
    return in_maps


def run(x, W_attn, b_attn, W_proj, b_proj, trace=False):
    nc = get_program()
    in_maps = make_in_maps(x, W_attn, b_attn, W_proj)
    res = bass_utils.run_bass_kernel_spmd(
        nc, in_maps, core_ids=list(range(N_CORES)), trace=trace
    )
    # combine: out[b] = sum_g outT_{2b+g}^T + (bv_g @ Wp_g summed) + b_proj
    corr = b_proj.astype(np.float64).copy()
    for g in range(2):
        bv_g = b_attn[2 * C + g * CL : 2 * C + (g + 1) * CL]
        corr += bv_g.astype(np.float64) @ W_proj[g * CL : (g + 1) * CL, :].astype(
            np.float64
        )
    out = np.empty((B, T, C), np.float32)
    for b in range(B):
        acc = (
            res.results[2 * b]["outT"].T.astype(np.float64)
            + res.results[2 * b + 1]["outT"].T.astype(np.float64)
            + corr
        )
        out[b] = acc.astype(np.float32)
    return out, res


def kernel(x, W_attn, b_attn, W_proj, b_proj):
    x = np.asarray(x, np.float32)
    W_attn = np.asarray(W_attn, np.float32)
    b_attn = np.asarray(b_attn, np.float32)
    W_proj = np.asarray(W_proj, np.float32)
    b_proj = np.asarray(b_proj, np.float32)
    out, _ = run(x, W_attn, b_attn, W_proj, b_proj)
    return out
